# revision 41
# baseline (speedup 1.0000x reference)
"""Trainium2 Bass kernel for nn_CrossAttention (B=4, S=1024, D=512, H=8).

Sharding: 8 cores = batch (4) x head-group (2 groups of 4 heads).
Each core computes a partial [S, E] output over its 256 feature dims;
the host sums the two partials per batch and adds the bias.

v3 design notes (per core; heads h=0..3, c=h//2, ho=64*(h%2)):
  - inputs shipped fp16; proj j-outer so PE overlaps the input DMA
  - kvT[c] [128f, 2048k] bf16; qT = kvT_l2r + kvT_r2l (proj linearity)
  - attention in 9 SLOTS per head, each exactly [128, 1024] of visible
    logits: slot 0 = l2r kb0, slots 1..7 = pair (l2r kb=s | r2l kb=s+7)
    packed back-to-back (widths sum to exactly 1024), slot 8 = r2l kb15.
    One exp per slot -> minimal ACT instruction count, zero wasted cols.
  - l2r diagonals land at slot-local col 0 and r2l diagonals at local 896
    for EVERY slot, so one strided gpsimd mul masks 4 slots at a time.
  - heads of a c-pair alternate per slot: their logits matmuls use row
    strips 0-63 / 64-127 (tile_position auto from base partition), which
    the PE runs concurrently and overlaps LDWEIGHTS across.
  - kva (k-major values, ones-augmented) via DMA-engine transposes of kvT
    blocks + one strided DVE copy per c -- no PE/DVE transpose cost.
  - temb: logits for 4 heads via col-tiled concurrent MMs; temb AV is a
    rank-1 update that doubles as the PSUM initializer (start=True).
  - normalize: copy xp->SBUF (frees PSUM early), then denom-row copy,
    PE ones-broadcast, reciprocal, mul -- all off the PE critical path.
  - PSUM: psA 2x[128,1024] (proj/logits/bc/outproj) + psX 2x[65,1024] = 8
"""

import os
import sys

sys.path.insert(0, "/opt/trn_rl_repo")

from contextlib import ExitStack

import numpy as np

import concourse.bass as bass
import concourse.mybir as mybir
import concourse.tile as tile
from concourse import bacc
from concourse.bass import ds, ts
from concourse.bass_utils import run_bass_kernel_spmd
from concourse.masks import make_identity


def _ensure_ntff_hook():
    """This image's antenv lacks axon_hooks; synthesize it so trace=True can
    reach the libaxon NTFF profiler (used by test.py, harmless otherwise)."""
    import types

    try:
        from antenv import axon_hooks  # noqa: F401

        return
    except ImportError:
        pass
    mod = types.ModuleType("antenv.axon_hooks")
    mod._hook = None
    mod.set_axon_ntff_profile_hook = lambda h: setattr(mod, "_hook", h)
    mod.get_axon_ntff_profile_hook = lambda: mod._hook
    import antenv

    sys.modules["antenv.axon_hooks"] = mod
    antenv.axon_hooks = mod
    try:
        from trn_agent_boot.trn_boot import _ntff_profile_via_ctypes

        mod._hook = _ntff_profile_via_ctypes("/opt/axon/libaxon_pjrt.so")
    except Exception:
        pass


_ensure_ntff_hook()

F32 = mybir.dt.float32
F16 = mybir.dt.float16
BF16 = mybir.dt.bfloat16
AF = mybir.ActivationFunctionType
ALU = mybir.AluOpType

P = 128
S = 1024
D = 512
E = 512
HG = 4  # heads per core
HD = 64
CS = HG * HD  # 256 feature cols per core
NKB = 16  # 8 l2r + 8 r2l key blocks (temb handled separately)
NSLOT = 9
SLOTW = 1024
EXW = NSLOT * SLOTW + 896  # slack so the last mask-batch rearrange fits


def _slot_parts(s):
    """Parts of slot s: (kb, q0, lcol0, w) with q range [q0, q0+w) at
    slot-local cols [lcol0, lcol0+w). kb 0..7 = l2r, 8..15 = r2l."""
    if s == 0:
        return [(0, 0, 0, 1024)]
    if s <= 7:
        w1 = 1024 - 128 * s
        return [(s, 128 * s, 0, w1), (s + 7, 0, w1, 128 * s)]
    return [(15, 0, 0, 1024)]


def _kv_block(kb):
    """kvT column range of key block kb."""
    return 128 * kb if kb < 8 else 1024 + 128 * (kb - 8)


def _qchunks(q0, w):
    """Split q range [q0, q0+w) at the global 512 boundary."""
    out = []
    if q0 < 512:
        out.append((q0, min(q0 + w, 512)))
    if q0 + w > 512:
        out.append((max(q0, 512), q0 + w))
    return out


def _build_body(ctx, tc):
    nc = tc.nc
    ctx.enter_context(
        nc.allow_low_precision(reason="bf16/fp16 matmul discipline")
    )

    xlT = nc.dram_tensor("xlT", [D, S], F16, kind="ExternalInput").ap()
    xrT = nc.dram_tensor("xrT", [D, S], F16, kind="ExternalInput").ap()
    tembT = nc.dram_tensor("tembT", [D, 2], F16, kind="ExternalInput").ap()
    wkT = nc.dram_tensor("wkT", [D, CS], F16, kind="ExternalInput").ap()
    woT = nc.dram_tensor("woT", [CS, E], BF16, kind="ExternalInput").ap()
    out = nc.dram_tensor("out_part", [S, E], F32, kind="ExternalOutput").ap()

    const = ctx.enter_context(tc.tile_pool(name="const", bufs=1))
    inp = ctx.enter_context(tc.tile_pool(name="inp", bufs=1))
    kvp = ctx.enter_context(tc.tile_pool(name="kvp", bufs=1))
    expp = ctx.enter_context(tc.tile_pool(name="expp", bufs=2))
    xts = ctx.enter_context(tc.tile_pool(name="xts", bufs=1))
    outp = ctx.enter_context(tc.tile_pool(name="outp", bufs=2))
    xpool = ctx.enter_context(tc.tile_pool(name="xpool", bufs=2))
    psA = ctx.enter_context(tc.tile_pool(name="psA", bufs=2, space="PSUM"))
    psX = ctx.enter_context(tc.tile_pool(name="psX", bufs=2, space="PSUM"))

    ident = const.tile([P, P], BF16)
    ident_stage = const.tile([P, P], F32)
    make_identity(nc, ident_stage[:])
    nc.vector.tensor_copy(ident[:], ident_stage[:])
    ones_bc = const.tile([P, HD], BF16)
    nc.gpsimd.memset(ones_bc[:], 1.0)
    # triangular 0/1 masks for the diagonal 128x128 of l2r / r2l key blocks
    ones_sq = const.tile([P, P], BF16)
    nc.gpsimd.memset(ones_sq[:], 1.0)
    mask_ut = const.tile([P, P], BF16)  # keep q >= k (l2r diag)
    nc.gpsimd.affine_select(
        mask_ut[:], ones_sq[:], pattern=[[1, P]], compare_op=ALU.is_ge,
        fill=0.0, base=0, channel_multiplier=-1,
    )
    mask_lt = const.tile([P, P], BF16)  # keep q <= k (r2l diag)
    nc.gpsimd.affine_select(
        mask_lt[:], ones_sq[:], pattern=[[-1, P]], compare_op=ALU.is_ge,
        fill=0.0, base=0, channel_multiplier=1,
    )

    # ---- input DMAs (fp16) split across both HWDGE queues (sync+scalar);
    # temb slivers first so the tiny temb projection starts immediately ----
    wk = inp.tile([P, 4, CS], F16)
    nc.sync.dma_start(out=wk[:], in_=wkT.rearrange("(c p) n -> p c n", p=P))
    allT = [inp.tile([P, 2 * S + 2], F16, name=f"allT{j}") for j in range(4)]
    for j in range(4):
        nc.scalar.dma_start(out=allT[j][:, 2 * S : 2 * S + 2], in_=tembT[ts(j, P), :])
    for j in range(4):
        nc.sync.dma_start(out=allT[j][:, 0:S], in_=xlT[ts(j, P), :])
        nc.scalar.dma_start(out=allT[j][:, S : 2 * S], in_=xrT[ts(j, P), :])
    wo = inp.tile([P, 2, E], BF16)
    nc.scalar.dma_start(out=wo[:], in_=woT.rearrange("(c p) n -> p c n", p=P))

    # ---- temb projection (tiny, runs first): kvtb[c][128f, 2] ----
    kvtb = [kvp.tile([P, 2], BF16, name=f"kvtb{c}") for c in range(2)]
    tas = []
    for c in range(2):
        ta = psA.tile([P, 2 * 512], F32, name="ta", tag="psA")
        for j in range(4):
            nc.tensor.matmul(
                ta[:, 0:2], wk[:, j, ts(c, P)], allT[j][:, ds(2 * S, 2)],
                start=(j == 0), stop=(j == 3),
            )
        tas.append(ta)
    for c in range(2):
        nc.vector.tensor_copy(kvtb[c][:], tas[c][:, 0:2])

    # ---- shared qkv projection: kvT[c][128f, 2048k] bf16 ----
    kvT = [kvp.tile([P, 2 * S], BF16, name=f"kvT{c}") for c in range(2)]
    qT = [kvp.tile([P, S], BF16, name=f"qT{c}") for c in range(2)]
    for c in range(2):
        pa = psA.tile([P, 2 * 512], F32, name="pa", tag="psA")
        pb = psA.tile([P, 2 * 512], F32, name="pb", tag="psA")
        for j in range(4):
            lw = wk[:, j, ts(c, P)]
            for n in range(4):
                dst = (pa if n < 2 else pb)[:, ds(512 * (n % 2), 512)]
                nc.tensor.matmul(
                    dst, lw, allT[j][:, ds(512 * n, 512)],
                    start=(j == 0), stop=(j == 3),
                )
        # casts on ACT (idle during proj), adds on GpSimd -- keeps DVE free
        nc.scalar.copy(kvT[c][:, 0:1024], pa[:])
        nc.scalar.copy(kvT[c][:, 1024:2048], pb[:])
    for c in range(2):
        nc.gpsimd.tensor_add(qT[c][:], kvT[c][:, 0:S], kvT[c][:, S : 2 * S])

    # ---- kv in k-major layout for AV via PE transposes ----
    # kva[c][128k, 2hp, 16kb, 65]: [.., 0:64] feats of head 2c+hp, [.., 64]=1
    kva = [kvp.tile([P, 2, NKB, HD + 1], BF16, name=f"kva{c}") for c in range(2)]
    for c in range(2):
        nc.vector.memset(kva[c][:, :, :, HD : HD + 1], 1.0)
        for g in range(2):  # 8 key blocks per staging tile
            tp = psA.tile([P, 8 * P], BF16, name="tp", tag="psA")
            for b in range(8):
                nc.tensor.transpose(
                    tp[:, ds(128 * b, P)],
                    kvT[c][:, ts(8 * g + b, P)],
                    ident[:],
                )
            # spread [k, kb, (hp f)] -> kva[k, hp, kb, f] in one strided copy
            nc.vector.tensor_copy(
                kva[c][:, :, ds(8 * g, 8), 0:HD],
                tp[:].rearrange("p (kb hp f) -> p hp kb f", kb=8, hp=2),
            )

    # kvta[128, 65]: row 32h = [temb-key feats of head h (64), 1.0]
    kvta = kvp.tile([P, HD + 1], BF16, name="kvta")
    nc.vector.memset(kvta[:], 0.0)
    nc.vector.memset(kvta[:, HD : HD + 1], 1.0)
    tpt = psA.tile([P, 2 * P], BF16, name="tpt", tag="psA")
    for c in range(2):
        nc.tensor.transpose(tpt[0:2, ds(128 * c, P)], kvtb[c][:], ident[:])
    for h in range(HG):
        c, hp = h // 2, h % 2
        nc.vector.tensor_copy(
            kvta[ds(32 * h, 1), 0:HD],
            tpt[0:1, ds(128 * c + 64 * hp, HD)],
        )

    # ---- temb logits for all 4 heads (col-tiled, concurrent) + exp ----
    TE = kvp.tile([P, S], BF16, name="TE")  # row 32h = exp temb logits head h
    tl = psA.tile([P, 2 * 512], F32, name="tl", tag="psA")
    nc.vector.memset(tl[:], 0.0)
    for h in range(HG):
        c, ho = h // 2, 64 * (h % 2)
        for qi in range(2):
            nc.tensor.matmul(
                tl[ds(32 * h, 1), ds(512 * qi, 512)],
                kvtb[c][ds(ho, HD), 0:1],
                qT[c][ds(ho, HD), ds(512 * qi, 512)],
                start=True, stop=True,
                tile_position=(ho, 32 * h),
            )
    nc.scalar.activation(TE[:], tl[:], AF.Exp, scale=0.125)

    # ---- attention ----
    xt2 = [xts.tile([P, S], BF16, name=f"xt2{c}") for c in range(2)]
    rec = kvp.tile([P, S], BF16, name="rec")
    if os.environ.get("KDBG", "0") == "1":
        nc.vector.memset(rec[:], 0.0)

    def emit_lg(h, s, EX):
        c, ho = h // 2, 64 * (h % 2)
        lg = psA.tile([P, 2 * 512], F32, name="lg", tag="psA")
        for kb, q0, l0, w in _slot_parts(s):
            kcol = _kv_block(kb)
            lc = 512 if (l0 < 512 < l0 + w) else None
            for a, b in ((l0, lc or l0 + w), (lc, l0 + w)) if lc else ((l0, l0 + w),):
                nc.tensor.matmul(
                    lg[:, ds(a, b - a)],
                    kvT[c][ds(ho, HD), ds(kcol, P)],
                    qT[c][ds(ho, HD), ds(q0 + a - l0, b - a)],
                    start=True, stop=True,
                )
        nc.scalar.activation(
            EX[:, ds(SLOTW * s, SLOTW)], lg[:], AF.Exp, scale=0.125
        )

    def emit_mask(kind, s0, nblk, EX):
        # diag of slot s sits at local col 0 (l2r/'ut') or 896 (r2l/'lt')
        c0 = SLOTW * s0 + (0 if kind == "ut" else 896)
        mt = mask_ut if kind == "ut" else mask_lt
        ex_ap = EX[:, ds(c0, nblk * SLOTW)].rearrange(
            "p (g x) -> p g x", g=nblk
        )[:, :, 0:P]
        m_ap, ex_ap2 = bass.broadcast_tensor_aps(
            mt[:].rearrange("p (g x) -> p g x", g=1), ex_ap
        )
        # split kinds across engines so the s==7 mask cluster (UT batch +
        # LT batch for both heads) doesn't serialize on one queue
        eng = nc.vector if kind == "ut" else nc.gpsimd
        eng.tensor_mul(ex_ap2, ex_ap2, m_ap)

    def emit_av(h, s, xp, EX):
        c, hp = h // 2, h % 2
        for kb, q0, l0, w in _slot_parts(s):
            for qa, qb in _qchunks(q0, w):
                nc.tensor.matmul(
                    xp[:, ds(qa, qb - qa)],
                    kva[c][:, hp, kb, :],
                    EX[:, ds(SLOTW * s + l0 + qa - q0, qb - qa)],
                    start=False, stop=(s == 8),
                    skip_group_check=True,
                )

    def emit_norm(h, xps):
        c, ho = h // 2, 64 * (h % 2)
        nc.vector.tensor_copy(rec[ds(32 * h, 1), :], xps[64:65, :])
        bc = psA.tile([P, 2 * 512], F32, name="bc", tag="psA")
        for qi in range(2):
            nc.tensor.matmul(
                bc[0:HD, ds(512 * qi, 512)],
                ones_bc[ds(32 * h, 1), :],
                rec[ds(32 * h, 1), ds(512 * qi, 512)],
                start=True, stop=True,
                tile_position=(32 * h, 0),
            )
        bcs = xpool.tile([HD, 2 * 512], F32, name="bcs", tag="bcs")
        nc.vector.reciprocal_approx_fast(bcs[:], bc[0:HD, :])
        nc.vector.tensor_mul(xt2[c][ds(ho, HD), :], xps[0:HD, :], bcs[:])

    norm_pend = []
    for cpair in range(2):
        hE, hO = 2 * cpair, 2 * cpair + 1
        xpE = psX.tile([65, 2 * 512], F32, name="xpE", tag="psX")
        xpO = psX.tile([65, 2 * 512], F32, name="xpO", tag="psX")
        EXE = expp.tile([P, EXW], BF16, name="EXE", tag="EX")
        EXO = expp.tile([P, EXW], BF16, name="EXO", tag="EX")
        for h, xp in ((hE, xpE), (hO, xpO)):
            for qi in range(2):
                nc.tensor.matmul(
                    xp[:, ds(512 * qi, 512)],
                    kvta[ds(32 * h, 1), :],
                    TE[ds(32 * h, 1), ds(512 * qi, 512)],
                    start=True, stop=False,
                    tile_position=(32 * h, 0),
                    skip_group_check=True,
                )
        for s in range(NSLOT):
            emit_lg(hE, s, EXE)
            emit_lg(hO, s, EXO)
            if s == 3:
                emit_mask("ut", 0, 4, EXE)
                emit_mask("ut", 0, 4, EXO)
            elif s == 4:
                emit_mask("lt", 1, 4, EXE)
                emit_mask("lt", 1, 4, EXO)
            elif s == 7:
                emit_mask("ut", 4, 4, EXE)
                emit_mask("ut", 4, 4, EXO)
                emit_mask("lt", 5, 3, EXE)
                emit_mask("lt", 5, 3, EXO)
            elif s == 8:
                emit_mask("lt", 8, 1, EXE)
                emit_mask("lt", 8, 1, EXO)
            if 4 <= s <= 7:
                emit_av(hE, s - 4, xpE, EXE)
                emit_av(hO, s - 4, xpO, EXO)
            elif s == 8:
                for sa in (4, 5, 6):
                    emit_av(hE, sa, xpE, EXE)
                    emit_av(hO, sa, xpO, EXO)
            if s == 2 and norm_pend:
                for hN, xpsN in norm_pend:
                    emit_norm(hN, xpsN)
                norm_pend = []
        for s in (7, 8):
            emit_av(hE, s, xpE, EXE)
            emit_av(hO, s, xpO, EXO)
        for h, xp in ((hE, xpE), (hO, xpO)):
            xps = xpool.tile([65, 2 * 512], F32, name="xps", tag="xps")
            if cpair == 0:  # ACT is saturated mid-attention, idle at the tail
                nc.vector.tensor_copy(xps[:], xp[:])
            else:
                nc.scalar.copy(xps[:], xp[:])
            if os.environ.get("KDBG", "0") == "1":
                dxp = nc.dram_tensor(
                    f"dbg_xps{h}", [65, 2 * 512], F32, kind="ExternalOutput"
                ).ap()
                nc.sync.dma_start(out=dxp, in_=xps[:])
            norm_pend.append((h, xps))
    for hN, xpsN in norm_pend:
        emit_norm(hN, xpsN)

    if os.environ.get("KDBG", "0") == "1":
        dbg = [
            ("kvT0", kvT[0][:], [P, 2 * S]), ("kvT1", kvT[1][:], [P, 2 * S]),
            ("qT0", qT[0][:], [P, S]), ("qT1", qT[1][:], [P, S]),
            ("xt20", xt2[0][:], [P, S]), ("xt21", xt2[1][:], [P, S]),
            ("TE", TE[:], [P, S]), ("kvta", kvta[:], [P, HD + 1]),
            ("rec", rec[:], [P, S]),
        ]
        for nm, ap, shp in dbg:
            dt_ = nc.dram_tensor(f"dbg_{nm}", shp, BF16, kind="ExternalOutput").ap()
            nc.sync.dma_start(out=dt_, in_=ap)

    # ---- output projection: out[s, e] = sum_c xt2[c].T @ wo[c] ----
    for st in range(8):
        pf = psA.tile([P, 2 * 512], F32, name="pf", tag="psA")
        for c in range(2):
            nc.tensor.matmul(
                pf[:, 0:E],
                xt2[c][:, ts(st, P)],
                wo[:, c, :],
                start=(c == 0), stop=(c == 1),
            )
        ob = outp.tile([P, E], F32, name="ob")
        if st % 2 == 0:
            nc.vector.tensor_copy(ob[:], pf[:, 0:E])
            nc.sync.dma_start(out=out[ts(st, P), :], in_=ob[:])
        else:
            nc.scalar.copy(ob[:], pf[:, 0:E])
            nc.scalar.dma_start(out=out[ts(st, P), :], in_=ob[:])


_NC_CACHE = None


def build_nc():
    global _NC_CACHE
    if _NC_CACHE is None:
        nc = bacc.Bacc(
            "TRN2",
            target_bir_lowering=False,
            debug=False,
            num_devices=8,
        )
        with tile.TileContext(nc) as tc, ExitStack() as ctx:
            _build_body(ctx, tc)
        nc.compile()
        _NC_CACHE = nc
    return _NC_CACHE


def make_in_maps(l2r_embed, r2l_embed, temb, W_dense, W_out):
    bf16 = mybir.dt.np(BF16)
    in_maps = []
    for core in range(8):
        b, hg = core // 2, core % 2
        cols = slice(CS * hg, CS * (hg + 1))
        tmb = np.zeros((D, 2), np.float16)
        tmb[:, 0] = temb[b].astype(np.float16)
        in_maps.append(
            {
                "xlT": np.ascontiguousarray(l2r_embed[b].T).astype(np.float16),
                "xrT": np.ascontiguousarray(r2l_embed[b].T).astype(np.float16),
                "tembT": tmb,
                "wkT": np.ascontiguousarray(W_dense[cols, :].T).astype(np.float16),
                "woT": np.ascontiguousarray(W_out[:, cols].T).astype(bf16),
            }
        )
    return in_maps


def kernel(l2r_embed, r2l_embed, temb, W_dense, W_out, b_out, num_heads, **run_kwargs):
    assert int(num_heads) == 8
    l2r_embed = np.asarray(l2r_embed, np.float32)
    r2l_embed = np.asarray(r2l_embed, np.float32)
    temb = np.asarray(temb, np.float32)
    W_dense = np.asarray(W_dense, np.float32)
    W_out = np.asarray(W_out, np.float32)
    b_out = np.asarray(b_out, np.float32)

    nc = build_nc()
    in_maps = make_in_maps(l2r_embed, r2l_embed, temb, W_dense, W_out)
    res = run_bass_kernel_spmd(nc, in_maps, core_ids=list(range(8)), **run_kwargs)

    B = l2r_embed.shape[0]
    outp = np.empty((B, S, E), np.float32)
    for b in range(B):
        outp[b] = (
            res.results[2 * b]["out_part"]
            + res.results[2 * b + 1]["out_part"]
            + b_out[None, :]
        )
    if run_kwargs:
        kernel.last_results = res
    return outp
